# revision 3
# baseline (speedup 1.0000x reference)
"""Causal self-attention on 8 TRN2 NeuronCores — v3.

Problem (hardcoded): B=4, T=2048, C=1024, H=16 heads, D=64.
  qkv = x @ W_in + b_in ; causal softmax attention ; out = y @ W_out + b_out

Sharding: core c handles batch b = c//2 and head-group g = c%2 (8 heads).
Each core computes its partial out-projection; the host adds the two
partials per batch plus b_out. No device collectives.

v3 design vs baseline:
  - S^T[k,q] matmuls in bf16 (qT/kT bf16), 128-granular diagonal skipping.
  - exp on ACT emits fp8e4m3 pT directly, with scale=1/8192 and bias=-1
    folded into the activation (W_qk and W_v are scaled x32 on host so
    fp8/bf16 quantization of weights stays in normal range).
  - PV flipped: psy[q, 65] += pT_pair^T @ V_pair with fp8 DoubleRow
    (0.5 cyc/row over 2 key-chunks = 4x fp32r).  V is split V8 + Vr
    (fp8 residual) for accuracy; ones column (=32) gives denominators
    and cancels the x32 weight scale in the softmax divide.
  - per-query softmax normalize: reciprocal[128,1] + tensor_scalar.
  - y_sb [q, (2 heads)*64] bf16 transposed by xbar DMA into yT[dh, q]
    for the bf16 out-projection.
"""

import sys

for _p in ("/opt/trn_rl_repo", "/root/.axon_site/_ro/trn_rl_repo"):
    if _p not in sys.path:
        sys.path.append(_p)

import ml_dtypes
import numpy as np

B, T, C = 4, 2048, 1024
H = 16  # total heads
HL = 8  # heads per core
D = 64  # head dim
P = 128
KO = C // P  # 8 contraction chunks
TQ = 512  # query-window width
NTQ = T // TQ  # 4 windows
FV = HL * D  # 512
DV = D + 1  # 64 d cols + ones col
SCL = 32.0  # host weight scale (fp8/bf16 friendliness)
EXPSCALE = 1.0 / (SCL * SCL * np.sqrt(D))  # undo q,k scaling + 1/sqrt(D)
EXPBIAS = -3.0  # exp(s-3): this float8e4 is IEEE e4m3 (max 240, has inf);
# scores reach ~8.1, exp(8.1-3)=164 leaves ~50% headroom

_CACHE = {}
DEBUG = False  # add kT/V8/Vr/yT debug outputs
COL0_SKIP = True  # skip fully-masked leading query columns of diag chunks
USE_DR = True  # fp8 DoubleRow PV (2 key-chunks per matmul)


def _build():
    import concourse.mybir as mybir
    import concourse.tile as tile
    from concourse import bacc

    fr = mybir.dt.float32r
    f32 = mybir.dt.float32
    bf16 = mybir.dt.bfloat16
    fp8 = mybir.dt.float8e4

    nc = bacc.Bacc("TRN2", target_bir_lowering=False, debug=False, num_devices=8)

    xT = nc.dram_tensor("xT", [C, T], fr, kind="ExternalInput")
    w_qk = nc.dram_tensor("w_qk", [C, 2 * HL * D], fr, kind="ExternalInput")
    b_qk = nc.dram_tensor("b_qk", [2 * HL * D], f32, kind="ExternalInput")
    w_v = nc.dram_tensor("w_v", [C, FV], fr, kind="ExternalInput")
    b_v = nc.dram_tensor("b_v", [FV], f32, kind="ExternalInput")
    w_out = nc.dram_tensor("w_out", [FV, C], bf16, kind="ExternalInput")
    ident = nc.dram_tensor("ident", [P, P], bf16, kind="ExternalInput")
    masks = nc.dram_tensor("masks", [P, 896], fp8, kind="ExternalInput")
    out = nc.dram_tensor("out", [T, C], f32, kind="ExternalOutput")
    if DEBUG:
        dbg_kT = nc.dram_tensor("dbg_kT", [NTQ, P, 4, TQ], bf16, kind="ExternalOutput")
        dbg_V8 = nc.dram_tensor("dbg_V8", [NTQ, P, 4, HL, DV], fp8, kind="ExternalOutput")
        dbg_Vr = nc.dram_tensor("dbg_Vr", [NTQ, P, 4, HL, DV], fp8, kind="ExternalOutput")
        dbg_yT = nc.dram_tensor("dbg_yT", [NTQ, P, 4, TQ], bf16, kind="ExternalOutput")
        dbg_psy = nc.dram_tensor("dbg_psy", [2, P, 4, P], f32, kind="ExternalOutput")
        dbg_ysb = nc.dram_tensor("dbg_ysb", [P, 4, 2, D], bf16, kind="ExternalOutput")
        dbg_pT = nc.dram_tensor("dbg_pT", [P, 2, 2, TQ], fp8, kind="ExternalOutput")

    FQK = 2 * HL * D  # 1024 (q block then k block)

    with tile.TileContext(nc) as tc:
        import contextlib
        from collections import deque

        ctx = contextlib.ExitStack()
        with ctx:
            persist = ctx.enter_context(tc.tile_pool(name="persist", bufs=1))
            qT_pool = ctx.enter_context(tc.tile_pool(name="qT", bufs=2))
            xT_pool = ctx.enter_context(tc.tile_pool(name="xT", bufs=1))
            pT_pool = ctx.enter_context(tc.tile_pool(name="pT", bufs=4))
            ysb_pool = ctx.enter_context(tc.tile_pool(name="ysb", bufs=4))
            rec_pool = ctx.enter_context(tc.tile_pool(name="rec", bufs=4))
            vtmp_pool = ctx.enter_context(tc.tile_pool(name="vtmp", bufs=2))
            yT_pool = ctx.enter_context(tc.tile_pool(name="yT", bufs=2))
            o_pool = ctx.enter_context(tc.tile_pool(name="o", bufs=2))

            # ---- weights + first x window, in first-use order ----
            # batched strided DMAs: HWDGE is a single exclusive device with
            # ~625ns per dma_start, so one big AP beats 8 small ones
            xq = xT.rearrange("(ko p) t -> p ko t", p=P)
            wq = w_qk.rearrange("(ko p) f -> p ko f", p=P)
            wqk_t = persist.tile([P, KO, FQK], fr)
            xT0 = xT_pool.tile([P, KO, TQ], fr, tag="xT", name="xT0")
            # startup: interleave per-ko so the ko-outer window-0 projection
            # can start as soon as chunk 0 lands
            for ko in range(KO):
                nc.sync.dma_start(wqk_t[:, ko : ko + 1], wq[:, ko : ko + 1])
                nc.gpsimd.dma_start(xT0[:, ko : ko + 1], xq[:, ko : ko + 1, 0:TQ])
            b_qk_sb = persist.tile([P, KO], f32)
            nc.sync.dma_start(b_qk_sb, b_qk.rearrange("(fo p) -> p fo", p=P))
            wv_t = persist.tile([P, KO, FV], fr)
            nc.sync.dma_start(wv_t, w_v.rearrange("(ko p) f -> p ko f", p=P))
            bv_bc = persist.tile([P, FV], f32)
            nc.sync.dma_start(bv_bc, b_v[None, :].to_broadcast((P, FV)))
            mask_sb = persist.tile([P, 896], fp8)
            nc.sync.dma_start(mask_sb, masks[:])
            ident_sb = persist.tile([P, P], bf16)
            nc.sync.dma_start(ident_sb, ident[:])
            expbias_sb = persist.tile([P, 1], f32)
            nc.vector.memset(expbias_sb, EXPBIAS)
            w_out_sb = persist.tile([P, 4, C], bf16)  # [p, do, n]
            nc.sync.dma_start(w_out_sb, w_out.rearrange("(do p) n -> p do n", p=P))

            # per-window persistent activations
            kT_w = []  # [p(2 heads d), j, TQ] bf16 per window
            V8_w = []  # [p(tokens), t4, h, DV] fp8
            Vr_w = []
            for w in range(NTQ):
                kT_w.append(persist.tile([P, 4, TQ], bf16, tag=f"kT{w}", name=f"kT{w}"))
                V8_w.append(persist.tile([P, 4, HL, DV], fp8, tag=f"V8{w}", name=f"V8{w}"))
                Vr_w.append(persist.tile([P, 4, HL, DV], fp8, tag=f"Vr{w}", name=f"Vr{w}"))
                # ones columns: V8 carries SCL (cancels x32 weight scale in
                # the softmax divide), Vr contributes nothing.
                nc.vector.memset(V8_w[w][:, :, :, D], SCL)
                nc.vector.memset(Vr_w[w][:, :, :, D], 0.0)

            # ---------------- unit builders ----------------
            def load_xT(w):
                t_ = xT_pool.tile([P, KO, TQ], fr, tag="xT")
                cols = slice(w * TQ, (w + 1) * TQ)
                nc.sync.dma_start(t_[:, 0:4], xq[:, 0:4, cols])
                nc.sync.dma_start(t_[:, 4:], xq[:, 4:, cols])
                return t_

            def proj_qk_unit(w, fo, qT_next, xTs):
                def emit():
                    ps = ps_pj.tile([P, TQ], f32, tag="pj")
                    for ko in range(KO):
                        nc.tensor.matmul(
                            ps,
                            wqk_t[:, ko, fo * P : (fo + 1) * P],
                            xTs[:, ko],
                            start=(ko == 0),
                            stop=(ko == KO - 1),
                        )
                    dst = qT_next[:, fo] if fo < 4 else kT_w[w][:, fo - 4]
                    nc.vector.tensor_scalar(
                        dst,
                        ps,
                        b_qk_sb[:, fo : fo + 1],
                        None,
                        mybir.AluOpType.add,
                    )

                return emit

            def proj_v_unit(w, t4, xTs):
                def emit():
                    ps = ps_pj.tile([P, FV], f32, tag="pj")
                    for ko in range(KO):
                        nc.tensor.matmul(
                            ps,
                            xTs[:, ko, t4 * P : (t4 + 1) * P],
                            wv_t[:, ko],
                            start=(ko == 0),
                            stop=(ko == KO - 1),
                        )
                    emit_v_split(w, t4, ps)

                return emit

            def emit_v_split(w, t4, psv):
                """psv [P tokens, FV] f32 -> V8/Vr fp8 with bias add.

                Keep ACT exp-only: the fp8 cast + residual run on the idle
                Pool engine (DVE does just the PSUM evac + bias).
                """
                tbf = vtmp_pool.tile([P, FV], bf16, tag="vtmp")
                nc.vector.tensor_tensor(tbf, psv, bv_bc, mybir.AluOpType.add)
                t3 = tbf.rearrange("p (h d) -> p h d", h=HL)
                nc.gpsimd.tensor_copy(V8_w[w][:, t4, :, :D], t3)
                nc.gpsimd.tensor_tensor(
                    Vr_w[w][:, t4, :, :D],
                    t3,
                    V8_w[w][:, t4, :, :D],
                    mybir.AluOpType.subtract,
                )

            def op_unit(tq, ts_, yT_win, scalar_copy=False, tail_psum=False):
                def emit():
                    t0 = tq * TQ + ts_ * P
                    o_sb = o_pool.tile([P, C], f32, tag="o")
                    for n in range(2):
                        if tail_psum:
                            ps = ps_s.tile([P, 512], f32, tag="ps_s", name="ps_o")
                        else:
                            ps = ps_pj.tile([P, 512], f32, tag="pj")
                        for do in range(4):
                            nc.tensor.matmul(
                                ps,
                                yT_win[:, do, ts_ * P : (ts_ + 1) * P],
                                w_out_sb[:, do, n * 512 : (n + 1) * 512],
                                start=(do == 0),
                                stop=(do == 3),
                            )
                        dst = o_sb[:, n * 512 : (n + 1) * 512]
                        if scalar_copy:
                            nc.scalar.copy(dst, ps)
                        else:
                            nc.vector.tensor_copy(dst, ps)
                    nc.sync.dma_start(out[t0 : t0 + P, :], o_sb)

                return emit

            # paced filler drain
            class Pacer:
                def __init__(self, fillers, total_slots, backload=1.0, reserve=0):
                    self.fillers = deque(fillers)
                    self.total = max(1, total_slots)
                    self.n = len(fillers)
                    self.slot = 0
                    self.done = 0
                    self.backload = backload
                    self.reserve = reserve

                def tick(self):
                    self.slot += 1
                    want = min(
                        int(self.n * (self.slot / self.total) ** self.backload),
                        self.n - self.reserve,
                    )
                    while self.done < want and self.fillers:
                        self.fillers.popleft()()
                        self.done += 1

                def drain(self):
                    while self.fillers:
                        self.fillers.popleft()()

            def att_j(tq, j, qT_cur, yT_win, pacer):
                """Head pair (2j, 2j+1): S^T in bf16, exp->fp8 pT pairs,
                DoubleRow PV into psy[q, DV] per (head, qblock)."""
                npairs = 2 * (tq + 1)
                qA = qT_cur[0:D, j, :]
                qB = qT_cur[D:P, j, :]
                # one 2KB PSUM bank (= one hw zero-region = one accumulation
                # group) per head; qb slots at 128-col offsets inside it
                psys = [
                    ps_y.tile([P, 4, P], f32, tag=f"psy_{h}", name=f"psy{h}")
                    for h in range(2)
                ]
                for m in range(npairs):
                    pT = pT_pool.tile([P, 2, 2, TQ], fp8, tag="pT")
                    for slot in range(2):
                        i = 2 * m + slot
                        win, kslot = i // 4, i % 4
                        i4 = i - 4 * tq
                        diag = 0 <= i4 < 4
                        # exp window start: pair-aligned so DoubleRow reads
                        # no stale bytes (slot1 of a diag pair extends down
                        # 128 cols; the mask zeroes that region)
                        col0 = P * (i4 - slot) if (diag and COL0_SKIP) else 0
                        w_ = TQ - col0
                        pss = ps_s.tile([P, 2, TQ], f32, tag="ps_s")
                        kslice = slice(kslot * P, (kslot + 1) * P)
                        nc.tensor.matmul(
                            pss[:, 0, col0:TQ],
                            kT_w[win][0:D, j, kslice],
                            qA[:, col0:TQ],
                            start=True,
                            stop=True,
                        )
                        nc.tensor.matmul(
                            pss[:, 1, col0:TQ],
                            kT_w[win][D:P, j, kslice],
                            qB[:, col0:TQ],
                            start=True,
                            stop=True,
                        )
                        nc.scalar.activation(
                            pT[:, slot, :, col0:TQ],
                            pss[:, :, col0:TQ],
                            mybir.ActivationFunctionType.Exp,
                            bias=expbias_sb[:],
                            scale=EXPSCALE,
                        )
                        if diag:
                            # zero cols [col0, 128*i4) + triangle in the
                            # next 128 cols; mask[p,u] = (u >= p+384)
                            off = 384 - P * i4
                            mw = P * i4 + P - col0
                            nc.vector.tensor_tensor(
                                pT[:, slot, :, col0 : col0 + mw],
                                pT[:, slot, :, col0 : col0 + mw],
                                mask_sb[:, off + col0 : off + col0 + mw]
                                .unsqueeze(1)
                                .to_broadcast((P, 2, mw)),
                                mybir.AluOpType.mult,
                            )
                        pacer.tick()
                    # PV: psy[q, DV] += pT_pair^T @ V_pair (fp8 DoubleRow)
                    win, t4 = (2 * m) // 4, (2 * m) % 4
                    dm = m - 2 * tq  # diag pair index (0 or 1) if >= 0
                    qb0 = 2 * dm if dm >= 0 else 0
                    if DEBUG and tq == 1 and j == 2 and m == 0:
                        nc.sync.dma_start(dbg_pT[:], pT)
                    for h in range(2):
                        for Vw, is_last_term in ((V8_w, False), (Vr_w, True)):
                            vpair = Vw[win][:, t4 : t4 + 2, 2 * j + h, :]
                            for qb in range(qb0, 4):
                                st = m == 0 and not is_last_term and qb == qb0
                                sp = m == npairs - 1 and is_last_term and qb == 3
                                if USE_DR:
                                    nc.tensor.matmul(
                                        psys[h][:, qb, :DV],
                                        pT[:, :, h, qb * P : (qb + 1) * P],
                                        vpair,
                                        start=st,
                                        stop=sp,
                                        perf_mode=mybir.MatmulPerfMode.DoubleRow,
                                    )
                                else:
                                    for sl in range(2):
                                        nc.tensor.matmul(
                                            psys[h][:, qb, :DV],
                                            pT[:, sl, h, qb * P : (qb + 1) * P],
                                            vpair[:, sl],
                                            start=(st and sl == 0),
                                            stop=(sp and sl == 1),
                                        )
                # normalize: y = psy[:, :D] * (1/denom) ; denom = psy[:, D]
                if DEBUG and tq == 1 and j == 2:
                    for h in range(2):
                        psy_sb = vtmp_pool.tile([P, 4, P], f32, tag=f"psydump{h}")
                        nc.vector.tensor_copy(psy_sb, psys[h])
                        nc.sync.dma_start(dbg_psy[h], psy_sb)
                y_sb = ysb_pool.tile([P, 4, 2, D], bf16, tag="ysb")
                for qb in range(4):
                    for h in range(2):
                        rec = rec_pool.tile([P, 1], f32, tag="rec")
                        nc.vector.reciprocal(rec, psys[h][:, qb, D : D + 1])
                        nc.vector.tensor_scalar(
                            y_sb[:, qb, h, :],
                            psys[h][:, qb, :D],
                            rec,
                            None,
                            mybir.AluOpType.mult,
                        )
                if DEBUG and tq == 1 and j == 2:
                    nc.sync.dma_start(dbg_ysb[:], y_sb)
                # PE transpose per qb: y_sb[:, qb] [q, (h d)] -> pst [dh, q],
                # then one DVE evac into yT (the xbar DMA transpose gave
                # deterministically corrupt data on hw; PE path is safe)
                pst = ps_t.tile([P, 4, P], bf16, tag="pst", name="pst")
                for qb in range(4):
                    nc.tensor.matmul(
                        pst[:, qb, :],
                        y_sb[:, qb],
                        ident_sb,
                        is_transpose=True,
                        start=(qb == 0),
                        stop=(qb == 3),
                    )
                nc.vector.tensor_copy(yT_win[:, j, :], pst)

            # ---------------- emission ----------------
            # window-0 projection: ko-outer so PE starts on the first chunks
            qT_cur = qT_pool.tile([P, 4, TQ], tag="qT", dtype=bf16)
            with tc.tile_pool(name="pj0", bufs=1, space="PSUM") as pj0:
                ps_fo = [
                    pj0.tile([P, TQ], f32, tag=f"pj0_{fo}", name=f"pj0_{fo}")
                    for fo in range(KO)
                ]
                for ko in range(KO):
                    for fo in range(KO):
                        nc.tensor.matmul(
                            ps_fo[fo],
                            wqk_t[:, ko, fo * P : (fo + 1) * P],
                            xT0[:, ko],
                            start=(ko == 0),
                            stop=(ko == KO - 1),
                        )
                for fo in range(KO):
                    dst = qT_cur[:, fo] if fo < 4 else kT_w[0][:, fo - 4]
                    nc.vector.tensor_scalar(
                        dst,
                        ps_fo[fo],
                        b_qk_sb[:, fo : fo + 1],
                        None,
                        mybir.AluOpType.add,
                    )
                for t4 in range(4):
                    psv = pj0.tile([P, FV], f32, tag=f"pj0_{t4}", name=f"pj0v_{t4}")
                    for ko in range(KO):
                        nc.tensor.matmul(
                            psv,
                            xT0[:, ko, t4 * P : (t4 + 1) * P],
                            wv_t[:, ko],
                            start=(ko == 0),
                            stop=(ko == KO - 1),
                        )
                    emit_v_split(0, t4, psv)
            ps_pj = ctx.enter_context(tc.tile_pool(name="ps_pj", bufs=1, space="PSUM"))
            ps_s = ctx.enter_context(tc.tile_pool(name="ps_s", bufs=2, space="PSUM"))
            ps_y = ctx.enter_context(tc.tile_pool(name="ps_y", bufs=1, space="PSUM"))
            ps_t = ctx.enter_context(tc.tile_pool(name="ps_t", bufs=1, space="PSUM"))

            yT_prev = None
            for tq in range(NTQ):
                fillers = []
                qT_next = None
                if tq + 1 < NTQ:
                    xTs = load_xT(tq + 1)
                    qT_next = qT_pool.tile([P, 4, TQ], tag="qT", dtype=bf16)
                    for fo in range(KO):
                        fillers.append(proj_qk_unit(tq + 1, fo, qT_next, xTs))
                    for t4 in range(4):
                        fillers.append(proj_v_unit(tq + 1, t4, xTs))
                if yT_prev is not None:
                    for ts_ in range(4):
                        fillers.append(op_unit(tq - 1, ts_, yT_prev))
                yT_win = yT_pool.tile([P, 4, TQ], tag="yT", dtype=bf16, name="yT_win")
                pacer = Pacer(
                    fillers,
                    total_slots=4 * 4 * (tq + 1),
                    backload=3.0 if tq == NTQ - 1 else 1.5,
                    reserve=0,
                )
                if tq == 0 and fillers:
                    pacer.fillers.popleft()()
                    pacer.done += 1
                for j in range(HL // 2):
                    att_j(tq, j, qT_cur, yT_win, pacer)
                pacer.drain()
                if DEBUG:
                    nc.sync.dma_start(dbg_yT[tq], yT_win)
                    nc.sync.dma_start(dbg_kT[tq], kT_w[tq])
                    nc.sync.dma_start(dbg_V8[tq], V8_w[tq])
                    nc.sync.dma_start(dbg_Vr[tq], Vr_w[tq])
                qT_cur = qT_next
                yT_prev = yT_win
            for ts_ in range(4):
                op_unit(NTQ - 1, ts_, yT_prev, scalar_copy=True, tail_psum=True)()

    nc.compile()
    return nc


def _get_nc():
    if "nc" not in _CACHE:
        _CACHE["nc"] = _build()
    return _CACHE["nc"]


def kernel(x, W_in, b_in, W_out, b_out):
    from concourse.bass_utils import run_bass_kernel_spmd

    x = np.asarray(x, dtype=np.float32)
    W_in = np.asarray(W_in, dtype=np.float32)
    b_in = np.asarray(b_in, dtype=np.float32)
    W_out = np.asarray(W_out, dtype=np.float32)
    b_out = np.asarray(b_out, dtype=np.float32)

    # causal mask master: M[p, u] = 1 if u >= p + 384
    u = np.arange(896)[None, :]
    p = np.arange(P)[:, None]
    mask = (u >= p + 384).astype(np.float32)

    in_maps = []
    for c in range(8):
        b, g = c // 2, c % 2
        qc = slice(g * HL * D, (g + 1) * HL * D)
        kc = slice(C + g * HL * D, C + (g + 1) * HL * D)
        vc = slice(2 * C + g * HL * D, 2 * C + (g + 1) * HL * D)
        w_qk = np.concatenate([W_in[:, qc], W_in[:, kc]], axis=1) * SCL
        b_qk = np.concatenate([b_in[qc], b_in[kc]]) * SCL
        in_maps.append(
            {
                "xT": np.ascontiguousarray(x[b].T),
                "w_qk": np.ascontiguousarray(w_qk),
                "b_qk": np.ascontiguousarray(b_qk),
                "w_v": np.ascontiguousarray(W_in[:, vc] * SCL),
                "b_v": np.ascontiguousarray(b_in[vc] * SCL),
                "w_out": np.ascontiguousarray(
                    W_out[g * HL * D : (g + 1) * HL * D, :]
                ).astype(ml_dtypes.bfloat16),
                "ident": np.eye(P, dtype=np.float32).astype(ml_dtypes.bfloat16),
                "masks": mask.astype(ml_dtypes.float8_e4m3),
            }
        )

    global _last_in_maps, _last_res
    _last_in_maps = in_maps
    nc = _get_nc()
    res = run_bass_kernel_spmd(nc, in_maps, list(range(8)))
    _last_res = res

    out = np.empty((B, T, C), np.float32)
    for b in range(B):
        out[b] = res.results[2 * b]["out"] + res.results[2 * b + 1]["out"] + b_out
    return out


if __name__ == "__main__":
    rng = np.random.default_rng(0)
    x = rng.standard_normal((B, T, C), dtype=np.float32)
    W_in = rng.standard_normal((C, 3 * C), dtype=np.float32) / np.sqrt(C)
    b_in = np.zeros(3 * C, np.float32)
    W_out = rng.standard_normal((C, C), dtype=np.float32) / np.sqrt(C)
    b_out = np.zeros(C, np.float32)
    y = kernel(x=x, W_in=W_in, b_in=b_in, W_out=W_out, b_out=b_out)
    print("ok", y.shape, y.dtype)


# revision 4
# speedup vs baseline: 1.0014x; 1.0014x over previous
"""Causal self-attention on 8 TRN2 NeuronCores — v3.

Problem (hardcoded): B=4, T=2048, C=1024, H=16 heads, D=64.
  qkv = x @ W_in + b_in ; causal softmax attention ; out = y @ W_out + b_out

Sharding: core c handles batch b = c//2 and head-group g = c%2 (8 heads).
Each core computes its partial out-projection; the host adds the two
partials per batch plus b_out. No device collectives.

v3 design vs baseline:
  - S^T[k,q] matmuls in bf16 (qT/kT bf16), 128-granular diagonal skipping.
  - exp on ACT emits fp8e4m3 pT directly, with scale=1/8192 and bias=-1
    folded into the activation (W_qk and W_v are scaled x32 on host so
    fp8/bf16 quantization of weights stays in normal range).
  - PV flipped: psy[q, 65] += pT_pair^T @ V_pair with fp8 DoubleRow
    (0.5 cyc/row over 2 key-chunks = 4x fp32r).  V is split V8 + Vr
    (fp8 residual) for accuracy; ones column (=32) gives denominators
    and cancels the x32 weight scale in the softmax divide.
  - per-query softmax normalize: reciprocal[128,1] + tensor_scalar.
  - y_sb [q, (2 heads)*64] bf16 transposed by xbar DMA into yT[dh, q]
    for the bf16 out-projection.
"""

import sys

for _p in ("/opt/trn_rl_repo", "/root/.axon_site/_ro/trn_rl_repo"):
    if _p not in sys.path:
        sys.path.append(_p)

import ml_dtypes
import numpy as np

B, T, C = 4, 2048, 1024
H = 16  # total heads
HL = 8  # heads per core
D = 64  # head dim
P = 128
KO = C // P  # 8 contraction chunks
TQ = 512  # query-window width
NTQ = T // TQ  # 4 windows
FV = HL * D  # 512
DV = D + 1  # 64 d cols + ones col
SCL = 32.0  # host weight scale (fp8/bf16 friendliness)
EXPSCALE = 1.0 / (SCL * SCL * np.sqrt(D))  # undo q,k scaling + 1/sqrt(D)
EXPBIAS = -3.0  # exp(s-3): this float8e4 is IEEE e4m3 (max 240, has inf);
# scores reach ~8.1, exp(8.1-3)=164 leaves ~50% headroom

_CACHE = {}
DEBUG = False  # add kT/V8/Vr/yT debug outputs
COL0_SKIP = True  # skip fully-masked leading query columns of diag chunks
USE_DR = True  # fp8 DoubleRow PV (2 key-chunks per matmul)


def _build():
    import concourse.mybir as mybir
    import concourse.tile as tile
    from concourse import bacc

    fr = mybir.dt.float32r
    f32 = mybir.dt.float32
    bf16 = mybir.dt.bfloat16
    fp8 = mybir.dt.float8e4

    nc = bacc.Bacc("TRN2", target_bir_lowering=False, debug=False, num_devices=8)

    xT = nc.dram_tensor("xT", [C, T], fr, kind="ExternalInput")
    w_qk = nc.dram_tensor("w_qk", [C, 2 * HL * D], fr, kind="ExternalInput")
    b_qk = nc.dram_tensor("b_qk", [2 * HL * D], f32, kind="ExternalInput")
    w_v = nc.dram_tensor("w_v", [C, FV], fr, kind="ExternalInput")
    b_v = nc.dram_tensor("b_v", [FV], f32, kind="ExternalInput")
    w_out = nc.dram_tensor("w_out", [FV, C], bf16, kind="ExternalInput")
    ident = nc.dram_tensor("ident", [P, P], bf16, kind="ExternalInput")
    masks = nc.dram_tensor("masks", [P, 896], fp8, kind="ExternalInput")
    out = nc.dram_tensor("out", [T, C], f32, kind="ExternalOutput")
    if DEBUG:
        dbg_kT = nc.dram_tensor("dbg_kT", [NTQ, P, 4, TQ], bf16, kind="ExternalOutput")
        dbg_V8 = nc.dram_tensor("dbg_V8", [NTQ, P, 4, HL, DV], fp8, kind="ExternalOutput")
        dbg_Vr = nc.dram_tensor("dbg_Vr", [NTQ, P, 4, HL, DV], fp8, kind="ExternalOutput")
        dbg_yT = nc.dram_tensor("dbg_yT", [NTQ, P, 4, TQ], bf16, kind="ExternalOutput")
        dbg_psy = nc.dram_tensor("dbg_psy", [2, P, 4, P], f32, kind="ExternalOutput")
        dbg_ysb = nc.dram_tensor("dbg_ysb", [P, 4, 2, D], bf16, kind="ExternalOutput")
        dbg_pT = nc.dram_tensor("dbg_pT", [P, 2, 2, TQ], fp8, kind="ExternalOutput")

    FQK = 2 * HL * D  # 1024 (q block then k block)

    with tile.TileContext(nc) as tc:
        import contextlib
        from collections import deque

        ctx = contextlib.ExitStack()
        with ctx:
            persist = ctx.enter_context(tc.tile_pool(name="persist", bufs=1))
            qT_pool = ctx.enter_context(tc.tile_pool(name="qT", bufs=2))
            xT_pool = ctx.enter_context(tc.tile_pool(name="xT", bufs=1))
            pT_pool = ctx.enter_context(tc.tile_pool(name="pT", bufs=4))
            ysb_pool = ctx.enter_context(tc.tile_pool(name="ysb", bufs=4))
            rec_pool = ctx.enter_context(tc.tile_pool(name="rec", bufs=4))
            vtmp_pool = ctx.enter_context(tc.tile_pool(name="vtmp", bufs=2))
            yT_pool = ctx.enter_context(tc.tile_pool(name="yT", bufs=2))
            o_pool = ctx.enter_context(tc.tile_pool(name="o", bufs=2))

            # ---- weights + first x window, in first-use order ----
            # batched strided DMAs: HWDGE is a single exclusive device with
            # ~625ns per dma_start, so one big AP beats 8 small ones
            xq = xT.rearrange("(ko p) t -> p ko t", p=P)
            wq = w_qk.rearrange("(ko p) f -> p ko f", p=P)
            wqk_t = persist.tile([P, KO, FQK], fr)
            xT0 = xT_pool.tile([P, KO, TQ], fr, tag="xT", name="xT0")
            # startup: interleave per-ko so the ko-outer window-0 projection
            # can start as soon as chunk 0 lands
            for ko in range(KO):
                nc.sync.dma_start(wqk_t[:, ko : ko + 1], wq[:, ko : ko + 1])
                nc.gpsimd.dma_start(xT0[:, ko : ko + 1], xq[:, ko : ko + 1, 0:TQ])
            b_qk_sb = persist.tile([P, KO], f32)
            nc.sync.dma_start(b_qk_sb, b_qk.rearrange("(fo p) -> p fo", p=P))
            wv_t = persist.tile([P, KO, FV], fr)
            nc.sync.dma_start(wv_t, w_v.rearrange("(ko p) f -> p ko f", p=P))
            bv_bc = persist.tile([P, FV], f32)
            nc.sync.dma_start(bv_bc, b_v[None, :].to_broadcast((P, FV)))
            mask_sb = persist.tile([P, 896], fp8)
            nc.sync.dma_start(mask_sb, masks[:])
            ident_sb = persist.tile([P, P], bf16)
            nc.sync.dma_start(ident_sb, ident[:])
            expbias_sb = persist.tile([P, 1], f32)
            nc.vector.memset(expbias_sb, EXPBIAS)
            w_out_sb = persist.tile([P, 4, C], bf16)  # [p, do, n]
            nc.sync.dma_start(w_out_sb, w_out.rearrange("(do p) n -> p do n", p=P))

            # per-window persistent activations
            kT_w = []  # [p(2 heads d), j, TQ] bf16 per window
            V8_w = []  # [p(tokens), t4, h, DV] fp8
            Vr_w = []
            for w in range(NTQ):
                kT_w.append(persist.tile([P, 4, TQ], bf16, tag=f"kT{w}", name=f"kT{w}"))
                V8_w.append(persist.tile([P, 4, HL, DV], fp8, tag=f"V8{w}", name=f"V8{w}"))
                Vr_w.append(persist.tile([P, 4, HL, DV], fp8, tag=f"Vr{w}", name=f"Vr{w}"))
                # ones columns: V8 carries SCL (cancels x32 weight scale in
                # the softmax divide), Vr contributes nothing.
                nc.vector.memset(V8_w[w][:, :, :, D], SCL)
                nc.vector.memset(Vr_w[w][:, :, :, D], 0.0)

            # ---------------- unit builders ----------------
            def load_xT(w):
                t_ = xT_pool.tile([P, KO, TQ], fr, tag="xT")
                cols = slice(w * TQ, (w + 1) * TQ)
                nc.sync.dma_start(t_[:, 0:4], xq[:, 0:4, cols])
                nc.sync.dma_start(t_[:, 4:], xq[:, 4:, cols])
                return t_

            def proj_qk_unit(w, fo, qT_next, xTs):
                def emit():
                    ps = ps_pj.tile([P, TQ], f32, tag="pj")
                    for ko in range(KO):
                        nc.tensor.matmul(
                            ps,
                            wqk_t[:, ko, fo * P : (fo + 1) * P],
                            xTs[:, ko],
                            start=(ko == 0),
                            stop=(ko == KO - 1),
                        )
                    dst = qT_next[:, fo] if fo < 4 else kT_w[w][:, fo - 4]
                    nc.vector.tensor_scalar(
                        dst,
                        ps,
                        b_qk_sb[:, fo : fo + 1],
                        None,
                        mybir.AluOpType.add,
                    )

                return emit

            def proj_v_unit(w, t4, xTs):
                def emit():
                    ps = ps_pj.tile([P, FV], f32, tag="pj")
                    for ko in range(KO):
                        nc.tensor.matmul(
                            ps,
                            xTs[:, ko, t4 * P : (t4 + 1) * P],
                            wv_t[:, ko],
                            start=(ko == 0),
                            stop=(ko == KO - 1),
                        )
                    emit_v_split(w, t4, ps)

                return emit

            def emit_v_split(w, t4, psv):
                """psv [P tokens, FV] f32 -> V8/Vr fp8 with bias add.

                Keep ACT exp-only: the fp8 cast + residual run on the idle
                Pool engine (DVE does just the PSUM evac + bias).
                """
                tbf = vtmp_pool.tile([P, FV], bf16, tag="vtmp")
                nc.vector.tensor_tensor(tbf, psv, bv_bc, mybir.AluOpType.add)
                t3 = tbf.rearrange("p (h d) -> p h d", h=HL)
                nc.gpsimd.tensor_copy(V8_w[w][:, t4, :, :D], t3)
                nc.gpsimd.tensor_tensor(
                    Vr_w[w][:, t4, :, :D],
                    t3,
                    V8_w[w][:, t4, :, :D],
                    mybir.AluOpType.subtract,
                )

            def op_unit(tq, ts_, yT_win, scalar_copy=False, tail_psum=False):
                def emit():
                    t0 = tq * TQ + ts_ * P
                    o_sb = o_pool.tile([P, C], f32, tag="o")
                    for n in range(2):
                        if tail_psum:
                            ps = ps_s.tile([P, 512], f32, tag="ps_s", name="ps_o")
                        else:
                            ps = ps_pj.tile([P, 512], f32, tag="pj")
                        for do in range(4):
                            nc.tensor.matmul(
                                ps,
                                yT_win[:, do, ts_ * P : (ts_ + 1) * P],
                                w_out_sb[:, do, n * 512 : (n + 1) * 512],
                                start=(do == 0),
                                stop=(do == 3),
                            )
                        dst = o_sb[:, n * 512 : (n + 1) * 512]
                        if scalar_copy:
                            nc.scalar.copy(dst, ps)
                        else:
                            nc.vector.tensor_copy(dst, ps)
                    nc.sync.dma_start(out[t0 : t0 + P, :], o_sb)

                return emit

            # paced filler drain
            class Pacer:
                def __init__(self, fillers, total_slots, backload=1.0, reserve=0):
                    self.fillers = deque(fillers)
                    self.total = max(1, total_slots)
                    self.n = len(fillers)
                    self.slot = 0
                    self.done = 0
                    self.backload = backload
                    self.reserve = reserve

                def tick(self):
                    self.slot += 1
                    want = min(
                        int(self.n * (self.slot / self.total) ** self.backload),
                        self.n - self.reserve,
                    )
                    while self.done < want and self.fillers:
                        self.fillers.popleft()()
                        self.done += 1

                def drain(self):
                    while self.fillers:
                        self.fillers.popleft()()

            def att_j(tq, j, qT_cur, yT_win, pacer):
                """Head pair (2j, 2j+1): S^T in bf16, exp->fp8 pT pairs,
                DoubleRow PV into psy[q, DV] per (head, qblock)."""
                npairs = 2 * (tq + 1)
                qA = qT_cur[0:D, j, :]
                qB = qT_cur[D:P, j, :]
                # one 2KB PSUM bank (= one hw zero-region = one accumulation
                # group) per head; qb slots at 128-col offsets inside it
                psys = [
                    ps_y.tile([P, 4, P], f32, tag=f"psy_{h}", name=f"psy{h}")
                    for h in range(2)
                ]
                for m in range(npairs):
                    pT = pT_pool.tile([P, 2, 2, TQ], fp8, tag="pT")
                    for slot in range(2):
                        i = 2 * m + slot
                        win, kslot = i // 4, i % 4
                        i4 = i - 4 * tq
                        diag = 0 <= i4 < 4
                        # exp window start: pair-aligned so DoubleRow reads
                        # no stale bytes (slot1 of a diag pair extends down
                        # 128 cols; the mask zeroes that region)
                        col0 = P * (i4 - slot) if (diag and COL0_SKIP) else 0
                        w_ = TQ - col0
                        pss = ps_s.tile([P, 2, TQ], f32, tag="ps_s")
                        kslice = slice(kslot * P, (kslot + 1) * P)
                        nc.tensor.matmul(
                            pss[:, 0, col0:TQ],
                            kT_w[win][0:D, j, kslice],
                            qA[:, col0:TQ],
                            start=True,
                            stop=True,
                        )
                        nc.tensor.matmul(
                            pss[:, 1, col0:TQ],
                            kT_w[win][D:P, j, kslice],
                            qB[:, col0:TQ],
                            start=True,
                            stop=True,
                        )
                        nc.scalar.activation(
                            pT[:, slot, :, col0:TQ],
                            pss[:, :, col0:TQ],
                            mybir.ActivationFunctionType.Exp,
                            bias=expbias_sb[:],
                            scale=EXPSCALE,
                        )
                        if diag:
                            # zero cols [col0, 128*i4) + triangle in the
                            # next 128 cols; mask[p,u] = (u >= p+384)
                            off = 384 - P * i4
                            mw = P * i4 + P - col0
                            nc.vector.tensor_tensor(
                                pT[:, slot, :, col0 : col0 + mw],
                                pT[:, slot, :, col0 : col0 + mw],
                                mask_sb[:, off + col0 : off + col0 + mw]
                                .unsqueeze(1)
                                .to_broadcast((P, 2, mw)),
                                mybir.AluOpType.mult,
                            )
                        pacer.tick()
                    # PV: psy[q, DV] += pT_pair^T @ V_pair (fp8 DoubleRow)
                    win, t4 = (2 * m) // 4, (2 * m) % 4
                    dm = m - 2 * tq  # diag pair index (0 or 1) if >= 0
                    qb0 = 2 * dm if dm >= 0 else 0
                    if DEBUG and tq == 1 and j == 2 and m == 0:
                        nc.sync.dma_start(dbg_pT[:], pT)
                    for h in range(2):
                        for Vw, is_last_term in ((V8_w, False), (Vr_w, True)):
                            vpair = Vw[win][:, t4 : t4 + 2, 2 * j + h, :]
                            for qb in range(qb0, 4):
                                st = m == 0 and not is_last_term and qb == qb0
                                sp = m == npairs - 1 and is_last_term and qb == 3
                                if USE_DR:
                                    nc.tensor.matmul(
                                        psys[h][:, qb, :DV],
                                        pT[:, :, h, qb * P : (qb + 1) * P],
                                        vpair,
                                        start=st,
                                        stop=sp,
                                        perf_mode=mybir.MatmulPerfMode.DoubleRow,
                                    )
                                else:
                                    for sl in range(2):
                                        nc.tensor.matmul(
                                            psys[h][:, qb, :DV],
                                            pT[:, sl, h, qb * P : (qb + 1) * P],
                                            vpair[:, sl],
                                            start=(st and sl == 0),
                                            stop=(sp and sl == 1),
                                        )
                # normalize: y = psy[:, :D] * (1/denom) ; denom = psy[:, D]
                if DEBUG and tq == 1 and j == 2:
                    for h in range(2):
                        psy_sb = vtmp_pool.tile([P, 4, P], f32, tag=f"psydump{h}")
                        nc.vector.tensor_copy(psy_sb, psys[h])
                        nc.sync.dma_start(dbg_psy[h], psy_sb)
                y_sb = ysb_pool.tile([P, 4, 2, D], bf16, tag="ysb")
                for qb in range(4):
                    for h in range(2):
                        rec = rec_pool.tile([P, 1], f32, tag="rec")
                        nc.vector.reciprocal(rec, psys[h][:, qb, D : D + 1])
                        nc.vector.tensor_scalar(
                            y_sb[:, qb, h, :],
                            psys[h][:, qb, :D],
                            rec,
                            None,
                            mybir.AluOpType.mult,
                        )
                if DEBUG and tq == 1 and j == 2:
                    nc.sync.dma_start(dbg_ysb[:], y_sb)
                # PE transpose per qb: y_sb[:, qb] [q, (h d)] -> pst [dh, q],
                # then one DVE evac into yT (the xbar DMA transpose gave
                # deterministically corrupt data on hw; PE path is safe)
                pst = ps_t.tile([P, 4, P], bf16, tag="pst", name="pst")
                for qb in range(4):
                    nc.tensor.matmul(
                        pst[:, qb, :],
                        y_sb[:, qb],
                        ident_sb,
                        is_transpose=True,
                        start=(qb == 0),
                        stop=(qb == 3),
                    )
                nc.vector.tensor_copy(yT_win[:, j, :], pst)

            # ---------------- emission ----------------
            # window-0 projection: ko-outer so PE starts on the first chunks
            qT_cur = qT_pool.tile([P, 4, TQ], tag="qT", dtype=bf16)
            with tc.tile_pool(name="pj0", bufs=1, space="PSUM") as pj0:
                ps_fo = [
                    pj0.tile([P, TQ], f32, tag=f"pj0_{fo}", name=f"pj0_{fo}")
                    for fo in range(KO)
                ]
                for ko in range(KO):
                    for fo in range(KO):
                        nc.tensor.matmul(
                            ps_fo[fo],
                            wqk_t[:, ko, fo * P : (fo + 1) * P],
                            xT0[:, ko],
                            start=(ko == 0),
                            stop=(ko == KO - 1),
                        )
                for fo in range(KO):
                    dst = qT_cur[:, fo] if fo < 4 else kT_w[0][:, fo - 4]
                    nc.vector.tensor_scalar(
                        dst,
                        ps_fo[fo],
                        b_qk_sb[:, fo : fo + 1],
                        None,
                        mybir.AluOpType.add,
                    )
                for t4 in range(4):
                    psv = pj0.tile([P, FV], f32, tag=f"pj0_{t4}", name=f"pj0v_{t4}")
                    for ko in range(KO):
                        nc.tensor.matmul(
                            psv,
                            xT0[:, ko, t4 * P : (t4 + 1) * P],
                            wv_t[:, ko],
                            start=(ko == 0),
                            stop=(ko == KO - 1),
                        )
                    emit_v_split(0, t4, psv)
            ps_pj = ctx.enter_context(tc.tile_pool(name="ps_pj", bufs=1, space="PSUM"))
            ps_s = ctx.enter_context(tc.tile_pool(name="ps_s", bufs=2, space="PSUM"))
            ps_y = ctx.enter_context(tc.tile_pool(name="ps_y", bufs=1, space="PSUM"))
            ps_t = ctx.enter_context(tc.tile_pool(name="ps_t", bufs=1, space="PSUM"))

            yT_prev = None
            for tq in range(NTQ):
                fillers = []
                qT_next = None
                if tq + 1 < NTQ:
                    xTs = load_xT(tq + 1)
                    qT_next = qT_pool.tile([P, 4, TQ], tag="qT", dtype=bf16)
                    for fo in range(KO):
                        fillers.append(proj_qk_unit(tq + 1, fo, qT_next, xTs))
                    for t4 in range(4):
                        fillers.append(proj_v_unit(tq + 1, t4, xTs))
                if yT_prev is not None:
                    for ts_ in range(4):
                        fillers.append(op_unit(tq - 1, ts_, yT_prev))
                yT_win = yT_pool.tile([P, 4, TQ], tag="yT", dtype=bf16, name="yT_win")
                pacer = Pacer(
                    fillers,
                    total_slots=4 * 4 * (tq + 1),
                    backload=3.0 if tq == NTQ - 1 else 2.0,
                    reserve=0,
                )
                if tq == 0 and fillers:
                    pacer.fillers.popleft()()
                    pacer.done += 1
                for j in range(HL // 2):
                    att_j(tq, j, qT_cur, yT_win, pacer)
                pacer.drain()
                if DEBUG:
                    nc.sync.dma_start(dbg_yT[tq], yT_win)
                    nc.sync.dma_start(dbg_kT[tq], kT_w[tq])
                    nc.sync.dma_start(dbg_V8[tq], V8_w[tq])
                    nc.sync.dma_start(dbg_Vr[tq], Vr_w[tq])
                qT_cur = qT_next
                yT_prev = yT_win
            for ts_ in range(4):
                op_unit(NTQ - 1, ts_, yT_prev, scalar_copy=True, tail_psum=True)()

    nc.compile()
    return nc


def _get_nc():
    if "nc" not in _CACHE:
        _CACHE["nc"] = _build()
    return _CACHE["nc"]


def kernel(x, W_in, b_in, W_out, b_out):
    from concourse.bass_utils import run_bass_kernel_spmd

    x = np.asarray(x, dtype=np.float32)
    W_in = np.asarray(W_in, dtype=np.float32)
    b_in = np.asarray(b_in, dtype=np.float32)
    W_out = np.asarray(W_out, dtype=np.float32)
    b_out = np.asarray(b_out, dtype=np.float32)

    # causal mask master: M[p, u] = 1 if u >= p + 384
    u = np.arange(896)[None, :]
    p = np.arange(P)[:, None]
    mask = (u >= p + 384).astype(np.float32)

    in_maps = []
    for c in range(8):
        b, g = c // 2, c % 2
        qc = slice(g * HL * D, (g + 1) * HL * D)
        kc = slice(C + g * HL * D, C + (g + 1) * HL * D)
        vc = slice(2 * C + g * HL * D, 2 * C + (g + 1) * HL * D)
        w_qk = np.concatenate([W_in[:, qc], W_in[:, kc]], axis=1) * SCL
        b_qk = np.concatenate([b_in[qc], b_in[kc]]) * SCL
        in_maps.append(
            {
                "xT": np.ascontiguousarray(x[b].T),
                "w_qk": np.ascontiguousarray(w_qk),
                "b_qk": np.ascontiguousarray(b_qk),
                "w_v": np.ascontiguousarray(W_in[:, vc] * SCL),
                "b_v": np.ascontiguousarray(b_in[vc] * SCL),
                "w_out": np.ascontiguousarray(
                    W_out[g * HL * D : (g + 1) * HL * D, :]
                ).astype(ml_dtypes.bfloat16),
                "ident": np.eye(P, dtype=np.float32).astype(ml_dtypes.bfloat16),
                "masks": mask.astype(ml_dtypes.float8_e4m3),
            }
        )

    global _last_in_maps, _last_res
    _last_in_maps = in_maps
    nc = _get_nc()
    res = run_bass_kernel_spmd(nc, in_maps, list(range(8)))
    _last_res = res

    out = np.empty((B, T, C), np.float32)
    for b in range(B):
        out[b] = res.results[2 * b]["out"] + res.results[2 * b + 1]["out"] + b_out
    return out


if __name__ == "__main__":
    rng = np.random.default_rng(0)
    x = rng.standard_normal((B, T, C), dtype=np.float32)
    W_in = rng.standard_normal((C, 3 * C), dtype=np.float32) / np.sqrt(C)
    b_in = np.zeros(3 * C, np.float32)
    W_out = rng.standard_normal((C, C), dtype=np.float32) / np.sqrt(C)
    b_out = np.zeros(C, np.float32)
    y = kernel(x=x, W_in=W_in, b_in=b_in, W_out=W_out, b_out=b_out)
    print("ok", y.shape, y.dtype)
